# revision 63
# baseline (speedup 1.0000x reference)
"""Multi-head causal attention with RoPE on 8 trn2 cores.

Sharding: core c -> batch b = c // 4, head group g = c % 4 (heads 4g..4g+4).
Each core computes q/k/v projections for its 4 heads, causal attention, and
a partial output-projection (its heads' slice of Wo). The host sums the 4
partials per batch (tensor-parallel unshard) and adds the output bias.

Device layout notes (v3 — PE-warmth + concurrency focused):
  - q/k use an interleaved head layout: tile A = local heads 0,1 / tile B =
    heads 2,3, each head on 64 consecutive partitions. QK^T is ONE K=64
    matmul per head per kk-chunk; heads alternate PE row strips (0,0)/(64,0)
    and are emitted back-to-back so pairs run concurrently in the 64-row
    tiled PE array.
  - RoPE: projection psum -> ACT drain c (+bias), a permutation matmul makes
    cswap (halves swapped per head), then q = c*COS + cswap*SSGN (3 DVE ops).
  - v keeps [s, d] layout with the softmax-denominator ones column FIRST per
    head, so the pv psum row 0 is the denominator at partition 0 (reciprocal
    straight from psum, gpsimd broadcast, one DVE multiply - no DMA hop).
  - Per-head score psums [128, 512] (1 bank) x 4-slab ring + 4 pv banks fill
    all 8 PSUM banks; exp ops are per-head on the Scalar engine.
  - Scheduling is built around the PE HAM clock-gate (idle gaps re-throttle
    the PE to 1.2 GHz): phases are interleaved so the PE never idles --
    v-proj groups fill the qk-proj ACT-drain windows, and o-proj groups for
    macro m-1 are spread through attention macro m (which is ACT-bound).
  - Input DMAs use >=1KB-per-partition descriptors (wq/wk packed 4-up into
    [128, 1024] tiles, wv packed 2-up into [128, 520]) and are ordered by
    first use, with wv2[j]/xT[j] interleaved so the first v-proj paces with
    DMA arrival; macro 0 carries the k-proj(1) tiles as PE fillers so the
    HAM clock-gate stays warm from the first attention macro on.
  - Output is stored fp16 (halves output DMA); host sums the partials.
"""

import os

import numpy as np
import ml_dtypes

BF16 = ml_dtypes.bfloat16

B, S, F = 2, 2048, 1024
H, D = 16, 64
HALF = D // 2
NCORES = 8
HPC = 4  # heads per core
S_TILES = S // 128  # 16
N_CH = S // 512  # 4  (512-wide column chunks of s)
F_CH = F // 128  # 8
MACROS = 4  # q macro tiles of 512
MAX_WAVELENGTH = 10000.0

_CACHE = {}
LAST_RESULT = None


def _build_nc():
    import concourse.bacc as bacc
    import concourse.tile as tile
    import concourse.mybir as mybir
    import concourse.bass as bass

    fp32 = mybir.dt.float32
    fp16 = mybir.dt.float16
    bf16 = mybir.dt.bfloat16
    MULT = mybir.AluOpType.mult
    EXP = mybir.ActivationFunctionType.Exp
    IDENT = mybir.ActivationFunctionType.Identity

    nc = bacc.Bacc("TRN2", target_bir_lowering=False, debug=False)

    xT_d = nc.dram_tensor("xT", [F + 1, S], bf16, kind="ExternalInput")
    wqk_d = nc.dram_tensor("wqk", [4, 128, 1024], bf16, kind="ExternalInput")
    wv2_d = nc.dram_tensor("wv2", [4, 128, 512], bf16, kind="ExternalInput")
    wo_d = nc.dram_tensor("wo", [256, F], bf16, kind="ExternalInput")
    bqk_d = nc.dram_tensor("bqk", [128, 4], fp32, kind="ExternalInput")
    cos_d = nc.dram_tensor("cosw", [128, S], bf16, kind="ExternalInput")
    ssgn_d = nc.dram_tensor("ssgnw", [128, S], bf16, kind="ExternalInput")
    perm_d = nc.dram_tensor("perm", [128, 128], bf16, kind="ExternalInput")
    mask_d = nc.dram_tensor("mask", [128, 128], bf16, kind="ExternalInput")
    outT_d = nc.dram_tensor("outT", [F, S], fp16, kind="ExternalOutput")

    with tile.TileContext(nc) as tc:
        with (
            tc.tile_pool(name="persist", bufs=1) as persist,
            tc.tile_pool(name="tmp", bufs=8) as tmp,
            tc.tile_pool(name="attn", bufs=8) as attn_pool,
            tc.tile_pool(name="ostage", bufs=6) as ostage,
            tc.tile_pool(name="ps", bufs=4, space="PSUM") as psA,
            tc.tile_pool(name="psPV", bufs=4, space="PSUM") as psPV,
        ):
            # ---------------- persistent SBUF tensors -------------------
            xT = [persist.tile([128, S], bf16, tag=f"xT{i}", name=f"xT{i}") for i in range(F_CH)]
            xones = persist.tile([1, S], bf16, tag="xones", name="xones")
            wqk = [persist.tile([128, 1024], bf16, tag=f"wqk{j}", name=f"wqk{j}") for j in range(4)]
            wv2 = [persist.tile([128, 512], bf16, tag=f"wv2{j}", name=f"wv2{j}") for j in range(4)]
            wo = [persist.tile([128, F], bf16, tag=f"wo{i}", name=f"wo{i}") for i in range(2)]
            bqk = persist.tile([128, 4], fp32, tag="bqk", name="bqk")
            cosw = persist.tile([128, S], bf16, tag="cosw", name="cosw")
            ssgnw = persist.tile([128, S], bf16, tag="ssgnw", name="ssgnw")
            permt = persist.tile([128, 128], bf16, tag="permt", name="permt")
            maskt = persist.tile([128, 128], bf16, tag="maskt", name="maskt")

            def wq_ap(kc):
                return wqk[kc % 4][:, 256 * (kc // 4) : 256 * (kc // 4) + 256]

            def wk_ap(kc):
                return wqk[kc % 4][:, 512 + 256 * (kc // 4) : 512 + 256 * (kc // 4) + 256]

            def wv_ap(kc):
                return wv2[kc % 4][:, 256 * (kc // 4) : 256 * (kc // 4) + 256]

            # post-RoPE q/k, interleaved layout
            qk_sb = {}
            for nm in ("qA", "qB", "kA", "kB"):
                qk_sb[nm] = persist.tile([128, S], bf16, tag=nm, name=nm)
            # v in [s, d] layout; head h: col 65h = ones, cols 65h+1..+65 = v
            v_sb = [persist.tile([128, 260], bf16, tag=f"v{i}", name=f"v{i}") for i in range(S_TILES)]
            # attention output, [dh, s] layout (head h -> tile h//2 rows 64*(h%2))
            aoT = [persist.tile([128, S], bf16, tag=f"aoT{i}", name=f"aoT{i}") for i in range(2)]

            # denominator ones-columns of v (col 65h per head), set once
            for st in range(S_TILES):
                ones_v = v_sb[st][:, :].rearrange("a (h c) -> a h c", h=4)[:, :, 0:1]
                nc.vector.memset(ones_v, 1.0)

            # ---------------- input DMA, ordered by first use -----------
            # wv2[j] and xT[j] interleaved so the v-proj contraction loop
            # paces with DMA arrival instead of waiting the whole set.
            for j in range(4):
                nc.sync.dma_start(out=wv2[j], in_=wv2_d[j])
                nc.sync.dma_start(out=xT[j][:, 0:512],
                                  in_=xT_d[128 * j : 128 * (j + 1), 0:512])
            for i in range(4, F_CH):
                nc.sync.dma_start(out=wqk[i - 4], in_=wqk_d[i - 4])
                nc.sync.dma_start(out=xT[i][:, 0:512], in_=xT_d[128 * i : 128 * (i + 1), 0:512])
            nc.sync.dma_start(out=xones, in_=xT_d[F : F + 1, :])
            nc.sync.dma_start(out=bqk, in_=bqk_d[:, :])
            nc.sync.dma_start(out=permt, in_=perm_d[:, :])
            nc.sync.dma_start(out=cosw, in_=cos_d[:, :])
            nc.sync.dma_start(out=ssgnw, in_=ssgn_d[:, :])
            nc.sync.dma_start(out=maskt, in_=mask_d[:, :])
            for i in range(F_CH):  # rest of x (3KB rows)
                nc.sync.dma_start(out=xT[i][:, 512:2048],
                                  in_=xT_d[128 * i : 128 * (i + 1), 512:2048])
            for i in range(2):
                nc.sync.dma_start(out=wo[i], in_=wo_d[128 * i : 128 * (i + 1), :])

            # ---------------- phase emitters ----------------------------
            def vproj(st):
                ps = psA.tile([128, 256], fp32, tag="ps", name="psv")
                sl = slice(128 * st, 128 * (st + 1))
                for kc in range(F_CH):
                    nc.tensor.matmul(ps, xT[kc][:, sl], wv_ap(kc),
                                     start=(kc == 0), stop=(kc == F_CH - 1))
                # strided drain skips the ones columns (set once by memset);
                # v bias is folded into the output bias on the host
                vv = v_sb[st][:, :].rearrange("a (h c) -> a h c", h=4)[:, :, 1:65]
                nc.scalar.copy(vv, ps[:, :].rearrange("a (h c) -> a h c", h=4))

            def qkproj_tile(which, n, ti):
                """Full chain (ps group, drain, perm, rope) for one tile --
                used as an attention-phase PE filler."""
                w_ap = wq_ap if which == "q" else wk_ap
                bcol = 0 if which == "q" else 2
                tn = "AB"[ti]
                nsl = slice(512 * n, 512 * (n + 1))
                p = psA.tile([128, 512], fp32, tag="ps", name="psp")
                for kc in range(F_CH):
                    nc.tensor.matmul(p, w_ap(kc)[:, 128 * ti : 128 * (ti + 1)],
                                     xT[kc][:, nsl],
                                     start=(kc == 0), stop=(kc == F_CH - 1))
                c = tmp.tile([128, 512], bf16, tag="rope", name="c")
                nc.scalar.activation(c, p, func=IDENT,
                                     bias=bqk[:, bcol + ti : bcol + ti + 1])
                pss = psA.tile([128, 512], fp32, tag="ps", name="pss")
                nc.tensor.matmul(pss, permt, c, start=True, stop=True)
                t1 = tmp.tile([128, 512], bf16, tag="rope", name="t1")
                t2 = tmp.tile([128, 512], bf16, tag="rope", name="t2")
                nc.vector.tensor_tensor(t2, pss, ssgnw[:, nsl], op=MULT)
                nc.vector.tensor_mul(t1, c, cosw[:, nsl])
                nc.vector.tensor_add(qk_sb[which + tn][:, nsl], t1, t2)

            def qkproj(which, n, vps=(None, None)):
                """Emission order ps_A, [vproj], ps_B, perm_A, [vproj],
                perm_B keeps the PE busy across the ACT drain latencies."""
                w_ap = wq_ap if which == "q" else wk_ap
                bcol = 0 if which == "q" else 2
                nsl = slice(512 * n, 512 * (n + 1))
                ps = {}
                for ti, tn in enumerate("AB"):
                    if ti == 1 and vps[0] is not None:
                        vproj(vps[0])
                    p = psA.tile([128, 512], fp32, tag="ps", name="psp")
                    for kc in range(F_CH):
                        nc.tensor.matmul(p, w_ap(kc)[:, 128 * ti : 128 * (ti + 1)],
                                         xT[kc][:, nsl],
                                         start=(kc == 0), stop=(kc == F_CH - 1))
                    ps[tn] = p
                cs = {}
                for ti, tn in enumerate("AB"):
                    c = tmp.tile([128, 512], bf16, tag="rope", name="c")
                    nc.scalar.activation(c, ps[tn], func=IDENT,
                                         bias=bqk[:, bcol + ti : bcol + ti + 1])
                    cs[tn] = c
                for ti, tn in enumerate("AB"):
                    if ti == 1 and vps[1] is not None:
                        vproj(vps[1])
                    out = qk_sb[which + tn]
                    pss = psA.tile([128, 512], fp32, tag="ps", name="pss")
                    nc.tensor.matmul(pss, permt, cs[tn], start=True, stop=True)
                    t1 = tmp.tile([128, 512], bf16, tag="rope", name="t1")
                    t2 = tmp.tile([128, 512], bf16, tag="rope", name="t2")
                    # t2 first: it reads the perm psum, freeing that slab
                    # for the next phase's score matmuls
                    nc.vector.tensor_tensor(t2, pss, ssgnw[:, nsl], op=MULT)
                    nc.vector.tensor_mul(t1, cs[tn], cosw[:, nsl])
                    nc.vector.tensor_add(out[:, nsl], t1, t2)

            def attn_macro(m, filler=None, split_norm=False):
                msl = slice(512 * m, 512 * (m + 1))
                nkk = 4 * m + 4
                pvT = [psPV.tile([65, 512], fp32, tag="pvT", name="pvT") for _ in range(HPC)]
                fidx = 0
                for kk in range(nkk):
                    t = kk - 4 * m  # >= 0 -> this kk-chunk holds the diagonal
                    lo = max(0, t) * 128
                    ksl = slice(128 * kk, 128 * (kk + 1))
                    qsl = slice(512 * m + lo, 512 * (m + 1))
                    # all 4 QK matmuls adjacent: heads alternate row strips
                    # (0,0)/(64,0) -> pairs run concurrent in the 64-row-
                    # tiled PE array.
                    sps, ats = [], []
                    for h in range(HPC):
                        if h == 2 and kk == 0 and filler is not None and fidx < len(filler):
                            # macro entry: heads 2,3 wait the tile-B rope
                            # chain on the backlogged DVE -- fill with an
                            # o-proj group
                            filler[fidx]()
                            fidx += 1
                        tn = "A" if h < 2 else "B"
                        band = slice(64 * (h % 2), 64 * (h % 2) + 64)
                        tp = (64 * (h % 2), 0)
                        ps = psA.tile([128, 512], fp32, tag="ps", name="sps")
                        nc.tensor.matmul(ps[:, lo:512], qk_sb["k" + tn][band, ksl],
                                         qk_sb["q" + tn][band, qsl],
                                         start=True, stop=True, tile_position=tp)
                        sps.append(ps)
                    for h in range(HPC):
                        at = attn_pool.tile([128, 512], bf16, tag="attn", name="at")
                        nc.scalar.activation(out=at[:, lo:512], in_=sps[h][:, lo:512],
                                             func=EXP, scale=0.125)
                        if t >= 0:
                            dsl = slice(128 * t, 128 * (t + 1))
                            nc.vector.tensor_tensor(at[:, dsl], at[:, dsl], maskt, op=MULT)
                        ats.append(at)
                    for h in range(HPC):
                        nc.tensor.matmul(pvT[h][:, lo:512],
                                         v_sb[kk][:, 65 * h : 65 * h + 65],
                                         ats[h][:, lo:512],
                                         start=(kk == 0), stop=(kk == nkk - 1))
                    # PE filler between chunks (ACT-bound phase)
                    if filler is not None and fidx < len(filler):
                        filler[fidx]()
                        fidx += 1
                if filler is not None:
                    for j in range(fidx, len(filler)):
                        filler[j]()
                # normalize: row 0 of pvT = denominator (ones-first v layout).
                # split_norm (last macro): 256-col halves so the rcp ->
                # broadcast -> mult -> DMA stages overlap, shortening the
                # tail before the final output projection.
                nh = 2 if split_norm else 1
                w = 512 // nh
                for h in range(HPC):
                    cix, r0 = h // 2, 64 * (h % 2)
                    for q in range(nh):
                        csl = slice(w * q, w * (q + 1))
                        osl = slice(512 * m + w * q, 512 * m + w * (q + 1))
                        rcp = tmp.tile([1, w], fp32, tag="rcp", name="rcp")
                        nc.vector.reciprocal_approx_fast(rcp, pvT[h][0:1, csl])
                        rb = tmp.tile([65, w], fp32, tag="rb", name="rb")
                        nc.gpsimd.partition_broadcast(rb, rcp[0:1, :])
                        ao = ostage.tile([65, w], bf16, tag="ao", name="ao")
                        nc.vector.tensor_tensor(ao, pvT[h][0:65, csl], rb, op=MULT)
                        nc.sync.dma_start(out=aoT[cix][r0 : r0 + 64, osl], in_=ao[1:65, :])

            def oproj_tail(sc):
                """Final s-chunk o-proj, split so the wo[0] half starts as
                soon as aoT[0] (heads 0,1) lands, overlapping the rest of
                the norm chain."""
                ssl = slice(512 * sc, 512 * (sc + 1))
                for half in range(2):
                    pws = []
                    for fo in range(4 * half, 4 * half + 4):
                        fsl = slice(128 * fo, 128 * (fo + 1))
                        pw = psA.tile([128, 512], fp32, tag="ps", name="pw")
                        nc.tensor.matmul(pw, wo[0][:, fsl], aoT[0][:, ssl],
                                         start=True, stop=False)
                        pws.append((fo, fsl, pw))
                    for fo, fsl, pw in pws:
                        nc.tensor.matmul(pw, wo[1][:, fsl], aoT[1][:, ssl],
                                         start=False, stop=True)
                        ow = ostage.tile([128, 512], fp16, tag="ow", name="ow")
                        nc.vector.tensor_copy(ow, pw)
                        nc.sync.dma_start(out=outT_d[fsl, ssl], in_=ow)

            def oproj_fo(sc, fo):
                ssl = slice(512 * sc, 512 * (sc + 1))
                fsl = slice(128 * fo, 128 * (fo + 1))
                pw = psA.tile([128, 512], fp32, tag="ps", name="pw")
                for c in range(2):
                    nc.tensor.matmul(pw, wo[c][:, fsl], aoT[c][:, ssl],
                                     start=(c == 0), stop=(c == 1))
                ow = ostage.tile([128, 512], fp16, tag="ow", name="ow")
                nc.vector.tensor_copy(ow, pw)
                nc.sync.dma_start(out=outT_d[fsl, ssl], in_=ow)

            # ---------------- schedule ----------------------------------
            # phase 0 does its v-projs upfront (DMA-gated, PE cold anyway);
            # phases 1-3 use their v-projs to fill qk-proj drain windows.
            for st in range(4):
                vproj(st)
            qkproj("k", 0)
            qkproj("q", 0)
            # attn(0) is ACT-bound with nothing else pending: k-proj(1)
            # tiles ride inside it as PE fillers (keeps HAM from
            # re-throttling on the macro-0 exp waits)
            attn_macro(0, filler=[lambda: qkproj_tile("k", 1, 0),
                                  lambda: qkproj_tile("k", 1, 1)])
            qkproj("q", 1, vps=(4, 5))
            vproj(6)
            vproj(7)
            attn_macro(1, filler=[(lambda f=f: oproj_fo(0, f)) for f in range(F_CH)])
            for n in range(2, N_CH):
                sts = list(range(4 * n, 4 * n + 4))
                qkproj("k", n, vps=(sts[0], sts[1]))
                qkproj("q", n, vps=(sts[2], sts[3]))
                fill = [(lambda sc=n - 1, fo=f: oproj_fo(sc, fo)) for f in range(F_CH)]
                attn_macro(n, filler=fill)
            oproj_tail(3)

    nc.compile()
    return nc


def _get_nc():
    if "nc" not in _CACHE:
        _CACHE["nc"] = _build_nc()
    return _CACHE["nc"]


def _host_prep(x, positions, Wq, bq, Wk, bk, Wv, bv, Wo, bo):
    """Build the 8 per-core input maps."""
    ts = MAX_WAVELENGTH ** (2.0 * np.arange(HALF, dtype=np.float32) / D)  # [32]
    ii = np.arange(128)
    mask = (ii[:, None] <= ii[None, :]).astype(BF16)
    perm = np.zeros((128, 128), dtype=BF16)
    src = (ii // 64) * 64 + (ii % 64 + 32) % 64
    perm[src, ii] = 1.0

    in_maps = []
    for c in range(NCORES):
        b, g = c // 4, c % 4
        heads = np.arange(4 * g, 4 * g + 4)

        xT = np.empty((F + 1, S), dtype=BF16)
        xT[:F] = x[b].T.astype(BF16)
        xT[F] = 1.0

        # v bias is folded into bo on the host (attn rows sum to 1)
        wv_e = Wv[:, 256 * g : 256 * (g + 1)].astype(BF16)
        wv2 = np.empty((4, 128, 512), dtype=BF16)
        for j in range(4):
            wv2[j, :, 0:256] = wv_e[128 * j : 128 * (j + 1)]
            wv2[j, :, 256:512] = wv_e[128 * (j + 4) : 128 * (j + 5)]

        csl = slice(256 * g, 256 * (g + 1))
        wq_c = Wq[:, csl].astype(BF16)
        wk_c = Wk[:, csl].astype(BF16)
        wqk = np.empty((4, 128, 1024), dtype=BF16)
        for j in range(4):
            wqk[j, :, 0:256] = wq_c[128 * j : 128 * (j + 1)]
            wqk[j, :, 256:512] = wq_c[128 * (j + 4) : 128 * (j + 5)]
            wqk[j, :, 512:768] = wk_c[128 * j : 128 * (j + 1)]
            wqk[j, :, 768:1024] = wk_c[128 * (j + 4) : 128 * (j + 5)]

        bqk = np.stack([bq[csl][:128], bq[csl][128:], bk[csl][:128], bk[csl][128:]],
                       axis=1).astype(np.float32)  # [128, 4]

        pos = positions[b].astype(np.float32)  # [S]
        ang = pos[None, :] / ts[:, None]  # [32, S]
        cos32, sin32 = np.cos(ang), np.sin(ang)
        cosw = np.tile(np.concatenate([cos32, cos32], 0), (2, 1)).astype(BF16)
        ssgnw = np.tile(np.concatenate([-sin32, sin32], 0), (2, 1)).astype(BF16)

        in_maps.append({
            "xT": xT,
            "wqk": wqk,
            "wv2": wv2,
            "wo": Wo[64 * heads[0] : 64 * heads[0] + 256, :].astype(BF16),
            "bqk": bqk,
            "cosw": cosw,
            "ssgnw": ssgnw,
            "perm": perm,
            "mask": mask,
        })
    return in_maps


def kernel(x, positions, Wq, bq, Wk, bk, Wv, bv, Wo, bo):
    global LAST_RESULT
    from concourse.bass_utils import run_bass_kernel_spmd

    x = np.asarray(x, dtype=np.float32)
    positions = np.asarray(positions)
    args = [np.asarray(a, dtype=np.float32) for a in (Wq, bq, Wk, bk, Wv, bv, Wo, bo)]
    Wq, bq, Wk, bk, Wv, bv, Wo, bo = args

    nc = _get_nc()
    in_maps = _host_prep(x, positions, Wq, bq, Wk, bk, Wv, bv, Wo, bo)
    try:
        res = run_bass_kernel_spmd(nc, in_maps, core_ids=list(range(NCORES)))
    except ModuleNotFoundError:
        # axon NTFF profiling hook unavailable in this image; run untraced
        os.environ["BASS_NEVER_TRACE"] = "1"
        res = run_bass_kernel_spmd(nc, in_maps, core_ids=list(range(NCORES)))
    LAST_RESULT = res

    bo_eff = bo + bv @ Wo  # v bias commutes through softmax-weighted sum
    out = np.empty((B, S, F), dtype=np.float32)
    for b in range(B):
        acc = np.zeros((F, S), dtype=np.float32)
        for g in range(4):
            acc += res.results[4 * b + g]["outT"].astype(np.float32)
        out[b] = acc.T + bo_eff[None, :]
    return out
